# revision 35
# baseline (speedup 1.0000x reference)
"""ALiBi attention-score kernel for 8 TRN2 NeuronCores.

Computes  out[b,h,i,j] = (q[b,h,i,:] * head_scales[h] / sqrt(D)) . k[b,h,j,:]
                         - slopes[h] * (pos[b,i] - pos[b,j])
with pos = positions[token_indices], for B=2, H=16, S=2048, D=128.

Sharding: the 32 (b,h) pairs are split 4-per-core across 8 cores (batch+head
parallel, no cross-core communication).

Device dataflow per (b,h):
 - PE: scores matmuls (K=128 bf16, N=512 chunks) into PSUM.
 - GpSimd: partition_broadcast builds R[p,j] = slope*pos_k[j] from a host
   f16 row (the +row part of the ALiBi bias).
 - Epilogue splits each q-tile's 2048 columns: the first V columns go
   through one fused VectorE op (affine_then_add: psum + colbias + R -> f16);
   the rest go ScalarE activation (psum + colbias) followed by a VectorE
   tensor_add of R in f16. Each q-tile uses two 2-bank PSUM tiles (4 in
   flight) so matmuls decouple from the epilogue.
 - f16 output tiles (1 MiB per 2 q-tiles) DMA'd to DRAM; host upcasts to f32.

Accuracy vs the f32 reference: ~4e-4 norm relative error (bf16 matmul +
f16 output rounding; the ALiBi bias terms are applied at >=f16 precision
against a bias-dominated output scale).
"""
import sys

if "/opt/trn_rl_repo" not in sys.path:
    sys.path.insert(0, "/opt/trn_rl_repo")


def _ensure_axon_hooks():
    """run_bass_kernel_spmd(trace=True) under axon imports antenv.axon_hooks,
    which this image lacks; provide a working stand-in so tracing (e.g. a
    harness setting BASS_TRACE) doesn't crash."""
    try:
        import antenv.axon_hooks  # noqa: F401
        return
    except ImportError:
        pass
    import types

    mod = types.ModuleType("antenv.axon_hooks")
    state = {"hook": None}
    try:
        import contextlib
        import ctypes

        lib = ctypes.CDLL("/opt/axon/libaxon_pjrt.so")
        if hasattr(lib, "axon_start_nrt_profile"):
            lib.axon_start_nrt_profile.argtypes = [
                ctypes.POINTER(ctypes.c_int64), ctypes.c_size_t]
            lib.axon_start_nrt_profile.restype = ctypes.c_int64
            lib.axon_stop_nrt_profile.argtypes = [ctypes.c_char_p]
            lib.axon_stop_nrt_profile.restype = ctypes.c_int64

            @contextlib.contextmanager
            def _hook(output_dir, device_ids):
                import jax

                jax.devices()
                if device_ids:
                    ids = (ctypes.c_int64 * len(device_ids))(*device_ids)
                    rc = lib.axon_start_nrt_profile(ids, len(device_ids))
                else:
                    rc = lib.axon_start_nrt_profile(None, 0)
                if rc != 0:
                    raise RuntimeError(f"axon_start_nrt_profile rc={rc}")
                try:
                    yield
                finally:
                    lib.axon_stop_nrt_profile(str(output_dir).encode())

            state["hook"] = _hook
    except Exception:
        pass

    mod.get_axon_ntff_profile_hook = lambda: state["hook"]
    mod.set_axon_ntff_profile_hook = lambda h: state.update(hook=h)
    sys.modules["antenv.axon_hooks"] = mod


_ensure_axon_hooks()

import math

import numpy as np
import ml_dtypes

import concourse.bacc as bacc
import concourse.mybir as mybir
import concourse.tile as tile
from concourse.bass_utils import run_bass_kernel_spmd

B, H, S, D = 2, 16, 2048, 128
N_CORES = 8
PAIRS_PER_CORE = (B * H) // N_CORES  # 4
QT = S // 128   # 16 q-tiles of 128 rows
NC_CHUNK = 512  # matmul free-dim (one PSUM bank)
NCH = S // NC_CHUNK  # 4
V_COLS = 448  # columns per q-tile via fused DVE affine_then_add; rest via ACT

BF16 = mybir.dt.bfloat16
F16 = mybir.dt.float16
F32 = mybir.dt.float32

_compiled_nc = None

# tunables overridable by the A/B bench harness
_CFG = {
    "v_cols": V_COLS,
    "split_head": True,
    "o_bufs": 4,
    "gp_cols": 0,  # trailing columns of the R-add handled by GpSimd
    "psum_split": True,  # two 2-bank psum tiles per q-tile instead of one 4-bank
    "o_qt": 2,            # q-tiles batched per output tile / DMA store
    "act_first": False,   # emit the ACT epilogue op before the DVE affine
    "in_bufs": 2,         # double-buffer depth for q/k/meta input tiles
    "warm": False,        # pull the ACT function-table load into the preamble
    "head2": False,       # smaller first-chunk loads for q-tile 0
    "pipe": True,         # one-q-tile-late R-add/store emission
    "asym": True,         # ps_lo = 1 bank (affine region), single-op ACT on ps_hi
    "hi_first": True,     # emit ps_hi matmul chunks before the ps_lo chunk
}


def _build_nc(**over):
    cfg = dict(_CFG)
    cfg.update(over)
    v_cols = cfg["v_cols"]
    split_head = cfg["split_head"]
    o_bufs = cfg["o_bufs"]
    gp_cols = cfg["gp_cols"]
    psum_split = cfg["psum_split"]
    o_qt = cfg["o_qt"]
    act_first = cfg["act_first"]
    in_bufs = cfg["in_bufs"]
    warm = cfg["warm"]
    head2 = cfg["head2"]
    pipe = cfg["pipe"]
    asym = cfg["asym"]
    hi_first = cfg["hi_first"]
    if asym:
        v_cols = NC_CHUNK
    nc = bacc.Bacc("TRN2", target_bir_lowering=False, debug=False,
                   num_devices=N_CORES)
    qT = nc.dram_tensor("qT", [PAIRS_PER_CORE, D, S], BF16, kind="ExternalInput")
    kT = nc.dram_tensor("kT", [PAIRS_PER_CORE, D, S], BF16, kind="ExternalInput")
    rrow = nc.dram_tensor("rrow", [PAIRS_PER_CORE, 1, S], F16,
                          kind="ExternalInput")
    pq = nc.dram_tensor("pq", [PAIRS_PER_CORE, 128, QT], F32,
                        kind="ExternalInput")
    out = nc.dram_tensor("out", [PAIRS_PER_CORE, S, S], F16,
                         kind="ExternalOutput")

    with tile.TileContext(nc) as tc:
        with (
            tc.tile_pool(name="qpool", bufs=in_bufs) as qpool,
            tc.tile_pool(name="kpool", bufs=in_bufs) as kpool,
            tc.tile_pool(name="spool", bufs=in_bufs) as spool,
            tc.tile_pool(name="rpool", bufs=in_bufs) as rpool,
            tc.tile_pool(name="opool", bufs=o_bufs) as opool,
            tc.tile_pool(name="psum",
                         bufs=(2 if asym else (4 if psum_split else 2)),
                         space="PSUM") as psum_pool,
        ):
            if warm:
                # tiny ACT op so its function-table DMA overlaps the input
                # loads instead of stalling q-tile 0's epilogue
                wt = spool.tile([1, 1], F32, tag="warm")
                nc.gpsimd.memset(wt[:], 0.0)
                nc.scalar.activation(wt[:], wt[:],
                                     mybir.ActivationFunctionType.Identity,
                                     bias=0.0, scale=1.0)

            for u in range(PAIRS_PER_CORE):
                q_t = qpool.tile([D, S], BF16, tag="q")
                k_t = kpool.tile([D, S], BF16, tag="k")
                rr_t = spool.tile([1, S], F16, tag="rr")
                pq_t = spool.tile([128, QT], F32, tag="pq")
                if u == 0 and head2:
                    # minimal first chunks: q-tile 0 lhsT + first rhs chunk
                    nc.sync.dma_start(q_t[:, 0:128], qT[u][:, 0:128])
                    nc.sync.dma_start(k_t[:, 0:NC_CHUNK], kT[u][:, 0:NC_CHUNK])
                    nc.sync.dma_start(k_t[:, NC_CHUNK:S], kT[u][:, NC_CHUNK:S])
                    nc.sync.dma_start(q_t[:, 128:S], qT[u][:, 128:S])
                elif u == 0 and split_head:
                    # split the first loads so q-tile 0's operands land early
                    nc.sync.dma_start(q_t[:, 0:256], qT[u][:, 0:256])
                    nc.sync.dma_start(k_t[:, 0:NC_CHUNK], kT[u][:, 0:NC_CHUNK])
                    nc.sync.dma_start(q_t[:, 256:S], qT[u][:, 256:S])
                    nc.sync.dma_start(k_t[:, NC_CHUNK:S], kT[u][:, NC_CHUNK:S])
                else:
                    nc.sync.dma_start(q_t[:], qT[u])
                    nc.sync.dma_start(k_t[:], kT[u])
                nc.sync.dma_start(rr_t[:], rrow[u])
                nc.sync.dma_start(pq_t[:], pq[u])

                # R[p, j] = slope*pos_k[j] replicated across partitions
                r16_t = rpool.tile([128, S], F16, tag="r16")
                nc.gpsimd.partition_broadcast(r16_t[:], rr_t[:])

                out_v = out[u].rearrange("(blk p) c -> p blk c", p=128)

                # epilogue part 2 (R-add on the ACT region + store) is
                # emitted one q-tile late so DVE never stalls on ACT
                pending = None
                for qt in range(QT):
                    if asym:
                        ps_lo = psum_pool.tile([128, NC_CHUNK], F32, tag="pslo")
                        ps_hi = psum_pool.tile([128, S - NC_CHUNK], F32,
                                               tag="pshi")
                    elif psum_split:
                        ps_lo = psum_pool.tile([128, S // 2], F32, tag="ps")
                        ps_hi = psum_pool.tile([128, S // 2], F32, tag="ps")
                    else:
                        ps = psum_pool.tile([128, S], F32, tag="ps")
                    chunk_order = [1, 2, 3, 0] if (asym and hi_first) \
                        else list(range(NCH))
                    for n in chunk_order:
                        sl = slice(n * NC_CHUNK, (n + 1) * NC_CHUNK)
                        if asym:
                            if n == 0:
                                dst = ps_lo[:, :]
                            else:
                                dst = ps_hi[:, (n - 1) * NC_CHUNK:n * NC_CHUNK]
                        elif psum_split:
                            t = ps_lo if n < NCH // 2 else ps_hi
                            off = (n % (NCH // 2)) * NC_CHUNK
                            dst = t[:, off:off + NC_CHUNK]
                        else:
                            dst = ps[:, sl]
                        nc.tensor.matmul(
                            dst,
                            q_t[:, qt * 128:(qt + 1) * 128],
                            k_t[:, sl],
                            start=True, stop=True,
                        )
                    if qt % o_qt == 0:
                        o16 = opool.tile([128, o_qt, S], F16, tag="o16")
                    half = qt % o_qt
                    colbias = pq_t[:, qt:qt + 1]

                    def emit_affine():
                        # fused: (psum + colbias) + R -> f16, first v_cols
                        src = ps_lo if psum_split else ps
                        nc.vector.affine_then_add(
                            o16[:, half, 0:v_cols], src[:, 0:v_cols],
                            r16_t[:, 0:v_cols], scale=1.0, bias=colbias,
                        )

                    def emit_act():
                        # remaining cols: ACT does psum + colbias
                        if asym:
                            nc.scalar.activation(
                                o16[:, half, NC_CHUNK:S], ps_hi[:, :],
                                mybir.ActivationFunctionType.Identity,
                                bias=colbias, scale=1.0,
                            )
                        elif psum_split:
                            nc.scalar.activation(
                                o16[:, half, v_cols:S // 2],
                                ps_lo[:, v_cols:S // 2],
                                mybir.ActivationFunctionType.Identity,
                                bias=colbias, scale=1.0,
                            )
                            nc.scalar.activation(
                                o16[:, half, S // 2:S], ps_hi[:, :],
                                mybir.ActivationFunctionType.Identity,
                                bias=colbias, scale=1.0,
                            )
                        else:
                            nc.scalar.activation(
                                o16[:, half, v_cols:S], ps[:, v_cols:S],
                                mybir.ActivationFunctionType.Identity,
                                bias=colbias, scale=1.0,
                            )

                    if act_first:
                        emit_act()
                        emit_affine()
                    else:
                        emit_affine()
                        emit_act()
                    def flush_pending(pending):
                        p_o16, p_half, p_store, p_r16 = pending
                        nc.vector.tensor_add(
                            p_o16[:, p_half, v_cols:S - gp_cols],
                            p_o16[:, p_half, v_cols:S - gp_cols],
                            p_r16[:, v_cols:S - gp_cols],
                        )
                        if gp_cols:
                            nc.gpsimd.tensor_add(
                                p_o16[:, p_half, S - gp_cols:S],
                                p_o16[:, p_half, S - gp_cols:S],
                                p_r16[:, S - gp_cols:S],
                            )
                        if p_store is not None:
                            nc.sync.dma_start(p_store, p_o16[:])

                    store = (out_v[:, qt - o_qt + 1:qt + 1, :]
                             if qt % o_qt == o_qt - 1 else None)
                    if pipe:
                        if pending is not None:
                            flush_pending(pending)
                        pending = (o16, half, store, r16_t)
                    else:
                        flush_pending((o16, half, store, r16_t))
                if pipe and pending is not None:
                    flush_pending(pending)
                    pending = None

    nc.compile()
    return nc


def _get_nc():
    global _compiled_nc
    if _compiled_nc is None:
        _compiled_nc = _build_nc()
    return _compiled_nc


def kernel(q, k, head_scales, slopes, positions, token_indices, **_unused):
    q = np.asarray(q, dtype=np.float32)
    k = np.asarray(k, dtype=np.float32)
    head_scales = np.asarray(head_scales, dtype=np.float32)
    slopes = np.asarray(slopes, dtype=np.float32)
    positions = np.asarray(positions, dtype=np.float32)
    token_indices = np.asarray(token_indices)

    base_scale = 1.0 / math.sqrt(D)
    pos = positions[token_indices]                              # [B, S] f32
    r = slopes[None, :, None] * pos[:, None, :]                 # [B, H, S] f32
    rrow = r.astype(np.float16)[:, :, None, :]                  # [B, H, 1, S]

    # pq[b,h,p,qt] = -r[b,h, qt*128+p]
    pq = -np.swapaxes(r.reshape(B, H, QT, 128), -1, -2)         # [B, H, 128, QT]
    pq = np.ascontiguousarray(pq)

    q_scaled = q * (head_scales * base_scale)[None, :, None, None]
    qT = np.ascontiguousarray(np.swapaxes(q_scaled, -1, -2)).astype(
        ml_dtypes.bfloat16)                                     # [B,H,D,S]
    kT = np.ascontiguousarray(np.swapaxes(k, -1, -2)).astype(
        ml_dtypes.bfloat16)                                     # [B,H,D,S]

    qT = qT.reshape(B * H, D, S)
    kT = kT.reshape(B * H, D, S)
    rrow = rrow.reshape(B * H, 1, S)
    pq = pq.reshape(B * H, 128, QT)

    in_maps = []
    for c in range(N_CORES):
        sl = slice(c * PAIRS_PER_CORE, (c + 1) * PAIRS_PER_CORE)
        in_maps.append({
            "qT": np.ascontiguousarray(qT[sl]),
            "kT": np.ascontiguousarray(kT[sl]),
            "rrow": np.ascontiguousarray(rrow[sl]),
            "pq": np.ascontiguousarray(pq[sl]),
        })

    nc = _get_nc()
    res = run_bass_kernel_spmd(nc, in_maps, core_ids=list(range(N_CORES)))
    outs = [np.asarray(res.results[c]["out"]) for c in range(N_CORES)]
    full = np.concatenate(outs, axis=0).reshape(B, H, S, S).astype(np.float32)
    return full


if __name__ == "__main__":
    rng = np.random.default_rng(0)
    inputs = {
        "q": rng.standard_normal((B, H, S, D), dtype=np.float32),
        "k": rng.standard_normal((B, H, S, D), dtype=np.float32),
        "head_scales": np.full((H,), 1.2, dtype=np.float32),
        "slopes": (2.0 ** (-8.0 * np.arange(1, H + 1) / H)).astype(np.float32),
        "positions": np.arange(S, dtype=np.float32),
        "token_indices": np.sort(rng.integers(0, S, (B, S)).astype(np.int32), axis=-1),
    }
    out = kernel(**inputs)
    print("kernel output", out.shape, out.dtype)
